# revision 11
# baseline (speedup 1.0000x reference)
"""MoE genre-gate kernel for 8 Trainium2 NeuronCores.

Strategy (expert-parallel with token dispatch, per sharding hint):
  - Routing (RMSNorm -> word+genre gate -> softmax -> top-2) is computed on
    host in float64: it is 0.03% of the FLOPs and produces the data-dependent
    dispatch tables (the stand-in for all-to-all).
  - The 8192 (token, expert) pairs are grouped per expert, each expert's
    token list split into chunks; chunks are ranked by size into
    SLOTS_PER_CORE capacity classes (one slot of each class per core, so
    every core has identical shapes = SPMD), and dispatched to the 8 cores.
  - Each core runs a dense 3-stage MLP over its expert slots in float32r
    (full PE rate at moving-dim >= 256, ~1.5e-4 matmul accuracy):
        h1 = relu(x @ W1 + b1); h2 = relu(h1 @ W2 + b2); y = (h2 @ W3) * cw
    with fp32 PSUM accumulation; cw is the top-k softmax combine weight
    (zero on padding rows, which makes padding contribute exactly 0).
  - Host scatter-adds the per-pair outputs back to [B,S,H] and adds the
    (cw @ b3) bias term.

Hardcoded problem shape: B=2, S=2048, H=1024, G=256, E=8, M=2048, top-2.
"""

import numpy as np

import concourse.bass as bass
import concourse.tile as tile
from concourse import mybir
from concourse.bass_utils import run_bass_kernel_spmd

TOP_K = 2
EPS = 1e-6
N_CORES = 8
SLOTS_PER_CORE = 3
H = 1024
M = 2048
KH, KM = H // 128, M // 128
F32R = mybir.dt.float32r
F32 = mybir.dt.float32


# ---------------------------------------------------------------------------
# walrus in this container accepts only ONE sync-wait command per
# instruction; Tile emits up to ~10.  Split extras onto standalone NoOps on
# the same engine, inserted immediately before the instruction, which
# preserves per-engine program order and therefore semantics.
_ctr = [0]


def _legalize_waits(nc, max_waits=1):
    for f in nc.m.functions:
        for blk in f.blocks:
            out = []
            for inst in blk.instructions:
                si = inst.sync_info
                if si is not None and len(si.on_wait) > max_waits:
                    waits = list(si.on_wait)
                    extra, keep = waits[:-max_waits], waits[-max_waits:]
                    for w in extra:
                        _ctr[0] += 1
                        out.append(mybir.InstNoOp(
                            name=f"waitsplit-{_ctr[0]}",
                            engine=inst.engine, ins=[], outs=[],
                            sync_info=mybir.SyncInfo(on_wait=[w], on_update=[]),
                        ))
                    inst.sync_info = mybir.SyncInfo(
                        on_wait=keep, on_update=list(si.on_update))
                out.append(inst)
            blk.instructions = out


# ---------------------------------------------------------------------------
def _route(x2d, genre_embed, rms_w, wg_W, wg_b, gg_W, gg_b, B, S):
    """Host gating in float64. Returns combine weights [T, E] (zero outside
    top-2)."""
    xd = x2d.astype(np.float64)
    var = np.mean(xd * xd, axis=-1, keepdims=True)
    xn = rms_w.astype(np.float64) * (xd / np.sqrt(var + EPS))
    gate = xn @ wg_W.astype(np.float64) + wg_b.astype(np.float64)
    gg = genre_embed.astype(np.float64)[:, 0, :] @ gg_W.astype(np.float64) \
        + gg_b.astype(np.float64)                       # [B, E]
    gate = gate.reshape(B, S, -1) + gg[:, None, :]
    gate = gate.reshape(B * S, -1)
    gate -= gate.max(axis=-1, keepdims=True)
    p = np.exp(gate)
    p /= p.sum(axis=-1, keepdims=True)
    top2 = np.argsort(-p, axis=-1)[:, :TOP_K]
    cw = np.zeros_like(p)
    rows = np.arange(p.shape[0])[:, None]
    cw[rows, top2] = p[rows, top2]
    return cw.astype(np.float32)


def _partition(counts):
    """Split experts into N_CORES*SLOTS_PER_CORE chunks (>=1 per expert,
    k_e ~ N_e, minimizing the max chunk), rank chunks into SLOTS_PER_CORE
    size classes of N_CORES chunks each, and give every core one chunk of
    each class (classes assigned so per-core totals equalize).

    Returns (CS, slots): CS[j] = capacity of class j (>=256);
    slots[core][j] = (expert, lo, hi) token range of that chunk."""
    E = len(counts)
    n_slots = N_CORES * SLOTS_PER_CORE
    k = [1] * E
    while sum(k) < n_slots:
        e = max(range(E), key=lambda i: -(-counts[i] // k[i]))
        k[e] += 1
    chunks = []
    for e in range(E):
        n, parts = counts[e], k[e]
        base, rem = divmod(n, parts)
        lo = 0
        for i in range(parts):
            sz = base + (1 if i < rem else 0)
            chunks.append((sz, e, lo, lo + sz))
            lo += sz
    chunks.sort(key=lambda c: -c[0])
    CS = []
    slots = [[None] * SLOTS_PER_CORE for _ in range(N_CORES)]
    for j in range(SLOTS_PER_CORE):
        cls = chunks[j * N_CORES:(j + 1) * N_CORES]
        sz = cls[0][0]
        CS.append(max(256, sz + (sz & 1)))
        order = sorted(range(N_CORES),
                       key=lambda c: sum(s[2] - s[1] for s in slots[c] if s))
        for i, (sz, e, lo, hi) in enumerate(sorted(cls, key=lambda c: -c[0])):
            slots[order[i]][j] = (e, lo, hi)
    return CS, slots


def _token_chunks(C):
    """Split C into matmul moving-dim chunks, each <=512 (PSUM bank) and as
    equal as possible (>=256 keeps float32r at full rate)."""
    assert C % 2 == 0
    n = -(-C // 512)
    h = C // 2
    base, rem = divmod(h, n)
    sizes = [2 * (base + (1 if i < rem else 0)) for i in range(n)]
    assert sum(sizes) == C and all(s <= 512 and s % 2 == 0 for s in sizes)
    return sizes


# ---------------------------------------------------------------------------
def _build_program(CS, legalize=True):
    """Emit the SPMD Bass program; CS = per-slot-class capacities."""
    S = len(CS)
    nc = bass.Bass()
    xt_d = [nc.dram_tensor(f"XT{s}", [H, CS[s]], F32R, kind="ExternalInput") for s in range(S)]
    w1_d = [nc.dram_tensor(f"W1{s}", [KM, 128, H], F32R, kind="ExternalInput") for s in range(S)]
    w2_d = [nc.dram_tensor(f"W2{s}", [KM, 128, M], F32R, kind="ExternalInput") for s in range(S)]
    w3_d = [nc.dram_tensor(f"W3{s}", [M, H], F32R, kind="ExternalInput") for s in range(S)]
    b1_d = [nc.dram_tensor(f"B1{s}", [M], F32, kind="ExternalInput") for s in range(S)]
    b2_d = [nc.dram_tensor(f"B2{s}", [M], F32, kind="ExternalInput") for s in range(S)]
    cw_d = [nc.dram_tensor(f"CW{s}", [CS[s]], F32, kind="ExternalInput") for s in range(S)]
    y_d = [nc.dram_tensor(f"Y{s}", [CS[s], H], F32, kind="ExternalOutput") for s in range(S)]

    HB = 512

    with tile.TileContext(nc) as tc:
        with (
            tc.tile_pool(name="xt", bufs=1) as p_xt,
            tc.tile_pool(name="w1", bufs=3) as p_w1,
            tc.tile_pool(name="w2", bufs=3) as p_w2,
            tc.tile_pool(name="w3", bufs=4) as p_w3,
            tc.tile_pool(name="h1", bufs=1) as p_h1,
            tc.tile_pool(name="h2", bufs=1) as p_h2,
            tc.tile_pool(name="bias", bufs=1) as p_b,
            tc.tile_pool(name="y", bufs=4) as p_y,
            tc.tile_pool(name="ps", bufs=8, space="PSUM") as p_ps,
        ):
            st = [dict() for _ in range(S)]   # per-slot tiles/geometry

            def emit_loads(s):
                C = CS[s]
                v = st[s]
                v["tcs"] = _token_chunks(C)
                v["tco"] = np.cumsum([0] + v["tcs"]).tolist()
                v["tts"] = [(i * 128, min(128, C - i * 128)) for i in range(-(-C // 128))]
                v["xt"] = [p_xt.tile([128, C], F32R, tag=f"xt{k}", name=f"xt_{s}_{k}") for k in range(KH)]
                for k in range(KH):
                    nc.sync.dma_start(out=v["xt"][k][:], in_=xt_d[s][k * 128:(k + 1) * 128, :])
                v["b1t"] = [p_b.tile([128, 1], F32, tag=f"b1_{s}_{m}", name=f"b1t_{s}_{m}") for m in range(KM)]
                v["b2t"] = [p_b.tile([128, 1], F32, tag=f"b2_{s}_{m}", name=f"b2t_{s}_{m}") for m in range(KM)]
                for m in range(KM):
                    nc.scalar.dma_start(out=v["b1t"][m][:], in_=b1_d[s][m * 128:(m + 1) * 128].rearrange("(p one) -> p one", one=1))
                    nc.scalar.dma_start(out=v["b2t"][m][:], in_=b2_d[s][m * 128:(m + 1) * 128].rearrange("(p one) -> p one", one=1))
                v["cwt"] = []
                for t, (t0, tn) in enumerate(v["tts"]):
                    v["cwt"].append(p_b.tile([tn, 1], F32, tag=f"cw_{s}_{t}", name=f"cwt_{s}_{t}"))
                    nc.scalar.dma_start(out=v["cwt"][t][:], in_=cw_d[s][t0:t0 + tn].rearrange("(p one) -> p one", one=1))
                v["h1"] = [p_h1.tile([128, C], F32R, tag=f"h1_{m}", name=f"h1_{s}_{m}") for m in range(KM)]
                v["h2"] = [p_h2.tile([128, C], F32R, tag=f"h2_{m}", name=f"h2_{s}_{m}") for m in range(KM)]

            def st1_group(s, m):
                v = st[s]
                w1t = p_w1.tile([128, H], F32R, tag="w1", name=f"w1t_{s}_{m}")
                nc.gpsimd.dma_start(out=w1t[:], in_=w1_d[s][m])
                for ci, tcz in enumerate(v["tcs"]):
                    ps = p_ps.tile([128, tcz], F32, tag="ps", name=f"ps1_{s}_{m}_{ci}")
                    for k in range(KH):
                        nc.tensor.matmul(
                            ps[:], w1t[:, k * 128:(k + 1) * 128],
                            v["xt"][k][:, v["tco"][ci]:v["tco"][ci + 1]],
                            start=(k == 0), stop=(k == KH - 1))
                    nc.scalar.activation(
                        v["h1"][m][:, v["tco"][ci]:v["tco"][ci + 1]], ps[:],
                        mybir.ActivationFunctionType.Relu, bias=v["b1t"][m][:, 0:1])

            def st2_group(s, m):
                v = st[s]
                w2t = p_w2.tile([128, M], F32R, tag="w2", name=f"w2t_{s}_{m}")
                nc.gpsimd.dma_start(out=w2t[:], in_=w2_d[s][m])
                for ci, tcz in enumerate(v["tcs"]):
                    ps = p_ps.tile([128, tcz], F32, tag="ps", name=f"ps2_{s}_{m}_{ci}")
                    for k in range(KM):
                        nc.tensor.matmul(
                            ps[:], w2t[:, k * 128:(k + 1) * 128],
                            v["h1"][k][:, v["tco"][ci]:v["tco"][ci + 1]],
                            start=(k == 0), stop=(k == KM - 1))
                    nc.scalar.activation(
                        v["h2"][m][:, v["tco"][ci]:v["tco"][ci + 1]], ps[:],
                        mybir.ActivationFunctionType.Relu, bias=v["b2t"][m][:, 0:1])

            def emit_w3(s, hb):
                w3t = [p_w3.tile([128, HB], F32R, tag=f"w3_{k % 4}", name=f"w3t_{s}_{hb}_{k}") for k in range(KM)]
                for k in range(KM):
                    nc.gpsimd.dma_start(
                        out=w3t[k][:],
                        in_=w3_d[s][k * 128:(k + 1) * 128, hb * HB:(hb + 1) * HB])
                st[s][f"w3_{hb}"] = w3t

            def st3_group(s, hb, t):
                v = st[s]
                t0, tn = v["tts"][t]
                w3t = v[f"w3_{hb}"]
                ps = p_ps.tile([tn, HB], F32, tag="ps", name=f"ps3_{s}_{hb}_{t}")
                for k in range(KM):
                    nc.tensor.matmul(
                        ps[:], v["h2"][k][:, t0:t0 + tn], w3t[k][:],
                        start=(k == 0), stop=(k == KM - 1))
                yt = p_y.tile([tn, HB], F32, tag="y", name=f"yt_{s}_{hb}_{t}")
                nc.scalar.activation(
                    yt[:], ps[:], mybir.ActivationFunctionType.Copy,
                    scale=v["cwt"][t][:, 0:1])
                nc.scalar.dma_start(
                    out=y_d[s][t0:t0 + tn, hb * HB:(hb + 1) * HB],
                    in_=yt[:])

            def st3_emitters(s):
                ems = []
                for hb in range(H // HB):
                    if hb > 0:
                        ems.append(lambda s=s, hb=hb: emit_w3(s, hb))
                    for t in range(len(st[s]["tts"])):
                        ems.append(lambda s=s, hb=hb, t=t: st3_group(s, hb, t))
                return ems

            def interleave(a_ems, b_ems):
                """Emit a and b emitter lists merged evenly (b spread among a)."""
                na, nb = len(a_ems), len(b_ems)
                bi = 0
                for i, a in enumerate(a_ems):
                    while bi < nb and bi * na <= i * nb:
                        b_ems[bi]()
                        bi += 1
                    a()
                while bi < nb:
                    b_ems[bi]()
                    bi += 1

            # ---- emission schedule: st3(s-1) interleaves with st1(s) ----
            emit_loads(0)
            prev_st3 = []
            for s in range(S):
                if s > 0:
                    emit_loads(s)
                interleave([lambda s=s, m=m: st1_group(s, m) for m in range(KM)],
                           prev_st3)
                emit_w3(s, 0)        # prefetch stage-3 hb=0 weights early
                for m in range(KM):
                    st2_group(s, m)
                prev_st3 = st3_emitters(s)
            for em in prev_st3:
                em()

    if legalize:
        _legalize_waits(nc)
    return nc


# ---------------------------------------------------------------------------
def kernel(x, genre_embed, rms_w, wg_W, wg_b, gg_W, gg_b, W1, b1, W2, b2, W3, b3):
    x = np.asarray(x, np.float32)
    B, S_, _ = x.shape
    T = B * S_
    x2d = np.ascontiguousarray(x.reshape(T, H))
    W1 = np.asarray(W1, np.float32)
    W2 = np.asarray(W2, np.float32)
    W3 = np.asarray(W3, np.float32)

    cw = _route(x2d, np.asarray(genre_embed, np.float32), np.asarray(rms_w, np.float32),
                np.asarray(wg_W, np.float32), np.asarray(wg_b, np.float32),
                np.asarray(gg_W, np.float32), np.asarray(gg_b, np.float32), B, S_)
    E = cw.shape[1]
    tok_by_e = [np.nonzero(cw[:, e])[0] for e in range(E)]
    counts = [len(t) for t in tok_by_e]
    CS, slots = _partition(counts)

    # pre-tile weights once per expert (shared across cores)
    used = set(e for core in slots for (e, _, _) in core)
    w1_tiled, w2_tiled = {}, {}
    for e in used:
        w1_tiled[e] = np.ascontiguousarray(
            W1[e].reshape(KH, 128, KM, 128).transpose(2, 1, 0, 3).reshape(KM, 128, H))
        w2_tiled[e] = np.ascontiguousarray(
            W2[e].reshape(KM, 128, KM, 128).transpose(2, 1, 0, 3).reshape(KM, 128, M))

    in_maps = []
    meta = []
    for core in range(N_CORES):
        im = {}
        cmeta = []
        for si, (e, lo, hi) in enumerate(slots[core]):
            C = CS[si]
            idx = tok_by_e[e][lo:hi]
            n = len(idx)
            xt = np.zeros((H, C), np.float32)
            xt[:, :n] = x2d[idx].T
            cwc = np.zeros((C,), np.float32)
            cwc[:n] = cw[idx, e]
            im[f"XT{si}"] = xt
            im[f"W1{si}"] = w1_tiled[e]
            im[f"W2{si}"] = w2_tiled[e]
            im[f"W3{si}"] = W3[e]
            im[f"B1{si}"] = np.asarray(b1[e], np.float32)
            im[f"B2{si}"] = np.asarray(b2[e], np.float32)
            im[f"CW{si}"] = cwc
            cmeta.append(idx)
        in_maps.append(im)
        meta.append(cmeta)

    nc = _build_program(CS)
    res = run_bass_kernel_spmd(nc, in_maps, list(range(N_CORES)))

    out2d = cw @ np.asarray(b3, np.float32)      # bias-3 combine term [T, H]
    for core in range(N_CORES):
        for si, idx in enumerate(meta[core]):
            y = res.results[core][f"Y{si}"]
            out2d[idx] += y[:len(idx)]
    return out2d.reshape(B, S_, H).astype(np.float32)


# revision 12
# speedup vs baseline: 1.1441x; 1.1441x over previous
"""MoE genre-gate kernel for 8 Trainium2 NeuronCores.

Strategy (expert-parallel with token dispatch, per sharding hint):
  - Routing (RMSNorm -> word+genre gate -> softmax -> top-2) is computed on
    host in float64: it is 0.03% of the FLOPs and produces the data-dependent
    dispatch tables (the stand-in for all-to-all).
  - The 8192 (token, expert) pairs are grouped per expert, each expert's
    token list split into chunks; chunks are ranked by size into
    SLOTS_PER_CORE capacity classes (one slot of each class per core, so
    every core has identical shapes = SPMD), and dispatched to the 8 cores.
  - Each core runs a dense 3-stage MLP over its expert slots in float32r
    (full PE rate at moving-dim >= 256, ~1.5e-4 matmul accuracy):
        h1 = relu(x @ W1 + b1); h2 = relu(h1 @ W2 + b2); y = (h2 @ W3) * cw
    with fp32 PSUM accumulation; cw is the top-k softmax combine weight
    (zero on padding rows, which makes padding contribute exactly 0).
  - Host scatter-adds the per-pair outputs back to [B,S,H] and adds the
    (cw @ b3) bias term.

Hardcoded problem shape: B=2, S=2048, H=1024, G=256, E=8, M=2048, top-2.
"""

import numpy as np

import concourse.bass as bass
import concourse.tile as tile
from concourse import mybir
from concourse.bass_utils import run_bass_kernel_spmd

TOP_K = 2
EPS = 1e-6
N_CORES = 8
SLOTS_PER_CORE = 3
H = 1024
M = 2048
KH, KM = H // 128, M // 128
F32R = mybir.dt.float32r
F32 = mybir.dt.float32


# ---------------------------------------------------------------------------
# walrus in this container accepts only ONE sync-wait command per
# instruction; Tile emits up to ~10.  Split extras onto standalone NoOps on
# the same engine, inserted immediately before the instruction, which
# preserves per-engine program order and therefore semantics.
_ctr = [0]


def _legalize_waits(nc, max_waits=1):
    for f in nc.m.functions:
        for blk in f.blocks:
            out = []
            for inst in blk.instructions:
                si = inst.sync_info
                if si is not None and len(si.on_wait) > max_waits:
                    waits = list(si.on_wait)
                    extra, keep = waits[:-max_waits], waits[-max_waits:]
                    for w in extra:
                        _ctr[0] += 1
                        out.append(mybir.InstNoOp(
                            name=f"waitsplit-{_ctr[0]}",
                            engine=inst.engine, ins=[], outs=[],
                            sync_info=mybir.SyncInfo(on_wait=[w], on_update=[]),
                        ))
                    inst.sync_info = mybir.SyncInfo(
                        on_wait=keep, on_update=list(si.on_update))
                out.append(inst)
            blk.instructions = out


# ---------------------------------------------------------------------------
def _route(x2d, genre_embed, rms_w, wg_W, wg_b, gg_W, gg_b, B, S):
    """Host gating in float64. Returns combine weights [T, E] (zero outside
    top-2)."""
    xd = x2d.astype(np.float64)
    var = np.mean(xd * xd, axis=-1, keepdims=True)
    xn = rms_w.astype(np.float64) * (xd / np.sqrt(var + EPS))
    gate = xn @ wg_W.astype(np.float64) + wg_b.astype(np.float64)
    gg = genre_embed.astype(np.float64)[:, 0, :] @ gg_W.astype(np.float64) \
        + gg_b.astype(np.float64)                       # [B, E]
    gate = gate.reshape(B, S, -1) + gg[:, None, :]
    gate = gate.reshape(B * S, -1)
    gate -= gate.max(axis=-1, keepdims=True)
    p = np.exp(gate)
    p /= p.sum(axis=-1, keepdims=True)
    top2 = np.argsort(-p, axis=-1)[:, :TOP_K]
    cw = np.zeros_like(p)
    rows = np.arange(p.shape[0])[:, None]
    cw[rows, top2] = p[rows, top2]
    return cw.astype(np.float32)


def _partition(counts):
    """Split experts into N_CORES*SLOTS_PER_CORE chunks (>=1 per expert,
    k_e ~ N_e, minimizing the max chunk), rank chunks into SLOTS_PER_CORE
    size classes of N_CORES chunks each, and give every core one chunk of
    each class (classes assigned so per-core totals equalize).

    Returns (CS, slots): CS[j] = capacity of class j (>=256);
    slots[core][j] = (expert, lo, hi) token range of that chunk."""
    E = len(counts)
    n_slots = N_CORES * SLOTS_PER_CORE
    k = [1] * E
    while sum(k) < n_slots:
        e = max(range(E), key=lambda i: -(-counts[i] // k[i]))
        k[e] += 1
    chunks = []
    for e in range(E):
        n, parts = counts[e], k[e]
        base, rem = divmod(n, parts)
        lo = 0
        for i in range(parts):
            sz = base + (1 if i < rem else 0)
            chunks.append((sz, e, lo, lo + sz))
            lo += sz
    chunks.sort(key=lambda c: -c[0])
    CS = []
    slots = [[None] * SLOTS_PER_CORE for _ in range(N_CORES)]
    for j in range(SLOTS_PER_CORE):
        cls = chunks[j * N_CORES:(j + 1) * N_CORES]
        sz = cls[0][0]
        CS.append(max(256, sz + (sz & 1)))
        order = sorted(range(N_CORES),
                       key=lambda c: sum(s[2] - s[1] for s in slots[c] if s))
        for i, (sz, e, lo, hi) in enumerate(sorted(cls, key=lambda c: -c[0])):
            slots[order[i]][j] = (e, lo, hi)
    return CS, slots


def _token_chunks(C):
    """Split C into matmul moving-dim chunks, each <=512 (PSUM bank) and as
    equal as possible (>=256 keeps float32r at full rate)."""
    assert C % 2 == 0
    n = -(-C // 512)
    h = C // 2
    base, rem = divmod(h, n)
    sizes = [2 * (base + (1 if i < rem else 0)) for i in range(n)]
    assert sum(sizes) == C and all(s <= 512 and s % 2 == 0 for s in sizes)
    return sizes


# ---------------------------------------------------------------------------
def _build_program(CS, legalize=True):
    """Emit the SPMD Bass program; CS = per-slot-class capacities."""
    S = len(CS)
    nc = bass.Bass()
    xt_d = [nc.dram_tensor(f"XT{s}", [H, CS[s]], F32R, kind="ExternalInput") for s in range(S)]
    w1_d = [nc.dram_tensor(f"W1{s}", [KM, 128, H], F32R, kind="ExternalInput") for s in range(S)]
    w2_d = [nc.dram_tensor(f"W2{s}", [KM, 128, M], F32R, kind="ExternalInput") for s in range(S)]
    w3_d = [nc.dram_tensor(f"W3{s}", [M, H], F32R, kind="ExternalInput") for s in range(S)]
    b1_d = [nc.dram_tensor(f"B1{s}", [M], F32, kind="ExternalInput") for s in range(S)]
    b2_d = [nc.dram_tensor(f"B2{s}", [M], F32, kind="ExternalInput") for s in range(S)]
    cw_d = [nc.dram_tensor(f"CW{s}", [CS[s]], F32, kind="ExternalInput") for s in range(S)]
    y_d = [nc.dram_tensor(f"Y{s}", [CS[s], H], F32, kind="ExternalOutput") for s in range(S)]

    HB = 512

    with tile.TileContext(nc) as tc:
        with (
            tc.tile_pool(name="xt", bufs=1) as p_xt,
            tc.tile_pool(name="w1", bufs=3) as p_w1,
            tc.tile_pool(name="w2", bufs=3) as p_w2,
            tc.tile_pool(name="w3", bufs=8) as p_w3,
            tc.tile_pool(name="h1", bufs=1) as p_h1,
            tc.tile_pool(name="h2", bufs=1) as p_h2,
            tc.tile_pool(name="bias", bufs=1) as p_b,
            tc.tile_pool(name="y", bufs=4) as p_y,
            tc.tile_pool(name="ps", bufs=8, space="PSUM") as p_ps,
        ):
            st = [dict() for _ in range(S)]   # per-slot tiles/geometry

            def emit_loads(s):
                C = CS[s]
                v = st[s]
                v["tcs"] = _token_chunks(C)
                v["tco"] = np.cumsum([0] + v["tcs"]).tolist()
                v["tts"] = [(i * 128, min(128, C - i * 128)) for i in range(-(-C // 128))]
                v["xt"] = [p_xt.tile([128, C], F32R, tag=f"xt{k}", name=f"xt_{s}_{k}") for k in range(KH)]
                for k in range(KH):
                    nc.sync.dma_start(out=v["xt"][k][:], in_=xt_d[s][k * 128:(k + 1) * 128, :])
                v["b1t"] = [p_b.tile([128, 1], F32, tag=f"b1_{s}_{m}", name=f"b1t_{s}_{m}") for m in range(KM)]
                v["b2t"] = [p_b.tile([128, 1], F32, tag=f"b2_{s}_{m}", name=f"b2t_{s}_{m}") for m in range(KM)]
                for m in range(KM):
                    nc.scalar.dma_start(out=v["b1t"][m][:], in_=b1_d[s][m * 128:(m + 1) * 128].rearrange("(p one) -> p one", one=1))
                    nc.scalar.dma_start(out=v["b2t"][m][:], in_=b2_d[s][m * 128:(m + 1) * 128].rearrange("(p one) -> p one", one=1))
                v["cwt"] = []
                for t, (t0, tn) in enumerate(v["tts"]):
                    v["cwt"].append(p_b.tile([tn, 1], F32, tag=f"cw_{s}_{t}", name=f"cwt_{s}_{t}"))
                    nc.scalar.dma_start(out=v["cwt"][t][:], in_=cw_d[s][t0:t0 + tn].rearrange("(p one) -> p one", one=1))
                v["h1"] = [p_h1.tile([128, C], F32R, tag=f"h1_{m}", name=f"h1_{s}_{m}") for m in range(KM)]
                v["h2"] = [p_h2.tile([128, C], F32R, tag=f"h2_{m}", name=f"h2_{s}_{m}") for m in range(KM)]

            def st1_group(s, m):
                v = st[s]
                w1t = p_w1.tile([128, H], F32R, tag="w1", name=f"w1t_{s}_{m}")
                nc.gpsimd.dma_start(out=w1t[:], in_=w1_d[s][m])
                for ci, tcz in enumerate(v["tcs"]):
                    ps = p_ps.tile([128, tcz], F32, tag="ps", name=f"ps1_{s}_{m}_{ci}")
                    for k in range(KH):
                        nc.tensor.matmul(
                            ps[:], w1t[:, k * 128:(k + 1) * 128],
                            v["xt"][k][:, v["tco"][ci]:v["tco"][ci + 1]],
                            start=(k == 0), stop=(k == KH - 1))
                    nc.scalar.activation(
                        v["h1"][m][:, v["tco"][ci]:v["tco"][ci + 1]], ps[:],
                        mybir.ActivationFunctionType.Relu, bias=v["b1t"][m][:, 0:1])

            def st2_group(s, m):
                v = st[s]
                w2t = p_w2.tile([128, M], F32R, tag="w2", name=f"w2t_{s}_{m}")
                nc.gpsimd.dma_start(out=w2t[:], in_=w2_d[s][m])
                for ci, tcz in enumerate(v["tcs"]):
                    ps = p_ps.tile([128, tcz], F32, tag="ps", name=f"ps2_{s}_{m}_{ci}")
                    for k in range(KM):
                        nc.tensor.matmul(
                            ps[:], w2t[:, k * 128:(k + 1) * 128],
                            v["h1"][k][:, v["tco"][ci]:v["tco"][ci + 1]],
                            start=(k == 0), stop=(k == KM - 1))
                    nc.scalar.activation(
                        v["h2"][m][:, v["tco"][ci]:v["tco"][ci + 1]], ps[:],
                        mybir.ActivationFunctionType.Relu, bias=v["b2t"][m][:, 0:1])

            def emit_w3(s, hb):
                w3t = [p_w3.tile([128, HB], F32R, tag=f"w3_{k % 4}", name=f"w3t_{s}_{hb}_{k}") for k in range(KM)]
                for k in range(KM):
                    nc.gpsimd.dma_start(
                        out=w3t[k][:],
                        in_=w3_d[s][k * 128:(k + 1) * 128, hb * HB:(hb + 1) * HB])
                st[s][f"w3_{hb}"] = w3t

            def st3_group(s, hb, t):
                v = st[s]
                t0, tn = v["tts"][t]
                w3t = v[f"w3_{hb}"]
                ps = p_ps.tile([tn, HB], F32, tag="ps", name=f"ps3_{s}_{hb}_{t}")
                for k in range(KM):
                    nc.tensor.matmul(
                        ps[:], v["h2"][k][:, t0:t0 + tn], w3t[k][:],
                        start=(k == 0), stop=(k == KM - 1))
                yt = p_y.tile([tn, HB], F32, tag="y", name=f"yt_{s}_{hb}_{t}")
                nc.scalar.activation(
                    yt[:], ps[:], mybir.ActivationFunctionType.Copy,
                    scale=v["cwt"][t][:, 0:1])
                nc.scalar.dma_start(
                    out=y_d[s][t0:t0 + tn, hb * HB:(hb + 1) * HB],
                    in_=yt[:])

            def st3_emitters(s):
                ems = []
                for hb in range(H // HB):
                    if hb > 0:
                        ems.append(lambda s=s, hb=hb: emit_w3(s, hb))
                    for t in range(len(st[s]["tts"])):
                        ems.append(lambda s=s, hb=hb, t=t: st3_group(s, hb, t))
                return ems

            def interleave(a_ems, b_ems):
                """Emit a and b emitter lists merged evenly (b spread among a)."""
                na, nb = len(a_ems), len(b_ems)
                bi = 0
                for i, a in enumerate(a_ems):
                    while bi < nb and bi * na <= i * nb:
                        b_ems[bi]()
                        bi += 1
                    a()
                while bi < nb:
                    b_ems[bi]()
                    bi += 1

            # ---- emission schedule: st3(s-1) interleaves with st1(s) ----
            emit_loads(0)
            prev_st3 = []
            for s in range(S):
                if s > 0:
                    emit_loads(s)
                interleave([lambda s=s, m=m: st1_group(s, m) for m in range(KM)],
                           prev_st3)
                emit_w3(s, 0)        # prefetch stage-3 hb=0 weights early
                for m in range(KM):
                    st2_group(s, m)
                prev_st3 = st3_emitters(s)
            for em in prev_st3:
                em()

    if legalize:
        _legalize_waits(nc)
    return nc


# ---------------------------------------------------------------------------
def kernel(x, genre_embed, rms_w, wg_W, wg_b, gg_W, gg_b, W1, b1, W2, b2, W3, b3):
    x = np.asarray(x, np.float32)
    B, S_, _ = x.shape
    T = B * S_
    x2d = np.ascontiguousarray(x.reshape(T, H))
    W1 = np.asarray(W1, np.float32)
    W2 = np.asarray(W2, np.float32)
    W3 = np.asarray(W3, np.float32)

    cw = _route(x2d, np.asarray(genre_embed, np.float32), np.asarray(rms_w, np.float32),
                np.asarray(wg_W, np.float32), np.asarray(wg_b, np.float32),
                np.asarray(gg_W, np.float32), np.asarray(gg_b, np.float32), B, S_)
    E = cw.shape[1]
    tok_by_e = [np.nonzero(cw[:, e])[0] for e in range(E)]
    counts = [len(t) for t in tok_by_e]
    CS, slots = _partition(counts)

    # pre-tile weights once per expert (shared across cores)
    used = set(e for core in slots for (e, _, _) in core)
    w1_tiled, w2_tiled = {}, {}
    for e in used:
        w1_tiled[e] = np.ascontiguousarray(
            W1[e].reshape(KH, 128, KM, 128).transpose(2, 1, 0, 3).reshape(KM, 128, H))
        w2_tiled[e] = np.ascontiguousarray(
            W2[e].reshape(KM, 128, KM, 128).transpose(2, 1, 0, 3).reshape(KM, 128, M))

    in_maps = []
    meta = []
    for core in range(N_CORES):
        im = {}
        cmeta = []
        for si, (e, lo, hi) in enumerate(slots[core]):
            C = CS[si]
            idx = tok_by_e[e][lo:hi]
            n = len(idx)
            xt = np.zeros((H, C), np.float32)
            xt[:, :n] = x2d[idx].T
            cwc = np.zeros((C,), np.float32)
            cwc[:n] = cw[idx, e]
            im[f"XT{si}"] = xt
            im[f"W1{si}"] = w1_tiled[e]
            im[f"W2{si}"] = w2_tiled[e]
            im[f"W3{si}"] = W3[e]
            im[f"B1{si}"] = np.asarray(b1[e], np.float32)
            im[f"B2{si}"] = np.asarray(b2[e], np.float32)
            im[f"CW{si}"] = cwc
            cmeta.append(idx)
        in_maps.append(im)
        meta.append(cmeta)

    nc = _build_program(CS)
    res = run_bass_kernel_spmd(nc, in_maps, list(range(N_CORES)))

    out2d = cw @ np.asarray(b3, np.float32)      # bias-3 combine term [T, H]
    for core in range(N_CORES):
        for si, idx in enumerate(meta[core]):
            y = res.results[core][f"Y{si}"]
            out2d[idx] += y[:len(idx)]
    return out2d.reshape(B, S_, H).astype(np.float32)


# revision 14
# speedup vs baseline: 1.1809x; 1.0321x over previous
"""MoE genre-gate kernel for 8 Trainium2 NeuronCores.

Strategy (expert-parallel with token dispatch, per sharding hint):
  - Routing (RMSNorm -> word+genre gate -> softmax -> top-2) is computed on
    host in float64: it is 0.03% of the FLOPs and produces the data-dependent
    dispatch tables (the stand-in for all-to-all).
  - The 8192 (token, expert) pairs are grouped per expert, each expert's
    token list split into chunks; chunks are ranked by size into
    SLOTS_PER_CORE capacity classes (one slot of each class per core, so
    every core has identical shapes = SPMD), and dispatched to the 8 cores.
  - Each core runs a dense 3-stage MLP over its expert slots in float32r
    (full PE rate at moving-dim >= 256, ~1.5e-4 matmul accuracy):
        h1 = relu(x @ W1 + b1); h2 = relu(h1 @ W2 + b2); y = (h2 @ W3) * cw
    with fp32 PSUM accumulation; cw is the top-k softmax combine weight
    (zero on padding rows, which makes padding contribute exactly 0).
  - Host scatter-adds the per-pair outputs back to [B,S,H] and adds the
    (cw @ b3) bias term.

Hardcoded problem shape: B=2, S=2048, H=1024, G=256, E=8, M=2048, top-2.
"""

import numpy as np

import concourse.bass as bass
import concourse.tile as tile
from concourse import mybir
from concourse.bass_utils import run_bass_kernel_spmd

TOP_K = 2
EPS = 1e-6
N_CORES = 8
SLOTS_PER_CORE = 2
H = 1024
M = 2048
KH, KM = H // 128, M // 128
F32R = mybir.dt.float32r
F32 = mybir.dt.float32


# ---------------------------------------------------------------------------
# walrus in this container accepts only ONE sync-wait command per
# instruction; Tile emits up to ~10.  Split extras onto standalone NoOps on
# the same engine, inserted immediately before the instruction, which
# preserves per-engine program order and therefore semantics.
_ctr = [0]


def _legalize_waits(nc, max_waits=1):
    for f in nc.m.functions:
        for blk in f.blocks:
            out = []
            for inst in blk.instructions:
                si = inst.sync_info
                if si is not None and len(si.on_wait) > max_waits:
                    waits = list(si.on_wait)
                    extra, keep = waits[:-max_waits], waits[-max_waits:]
                    for w in extra:
                        _ctr[0] += 1
                        out.append(mybir.InstNoOp(
                            name=f"waitsplit-{_ctr[0]}",
                            engine=inst.engine, ins=[], outs=[],
                            sync_info=mybir.SyncInfo(on_wait=[w], on_update=[]),
                        ))
                    inst.sync_info = mybir.SyncInfo(
                        on_wait=keep, on_update=list(si.on_update))
                out.append(inst)
            blk.instructions = out


# ---------------------------------------------------------------------------
def _route(x2d, genre_embed, rms_w, wg_W, wg_b, gg_W, gg_b, B, S):
    """Host gating in float64. Returns combine weights [T, E] (zero outside
    top-2)."""
    xd = x2d.astype(np.float64)
    var = np.mean(xd * xd, axis=-1, keepdims=True)
    xn = rms_w.astype(np.float64) * (xd / np.sqrt(var + EPS))
    gate = xn @ wg_W.astype(np.float64) + wg_b.astype(np.float64)
    gg = genre_embed.astype(np.float64)[:, 0, :] @ gg_W.astype(np.float64) \
        + gg_b.astype(np.float64)                       # [B, E]
    gate = gate.reshape(B, S, -1) + gg[:, None, :]
    gate = gate.reshape(B * S, -1)
    gate -= gate.max(axis=-1, keepdims=True)
    p = np.exp(gate)
    p /= p.sum(axis=-1, keepdims=True)
    top2 = np.argsort(-p, axis=-1)[:, :TOP_K]
    cw = np.zeros_like(p)
    rows = np.arange(p.shape[0])[:, None]
    cw[rows, top2] = p[rows, top2]
    return cw.astype(np.float32)


def _partition(counts):
    """Split experts into N_CORES*SLOTS_PER_CORE chunks (>=1 per expert,
    k_e ~ N_e, minimizing the max chunk), rank chunks into SLOTS_PER_CORE
    size classes of N_CORES chunks each, and give every core one chunk of
    each class (classes assigned so per-core totals equalize).

    Returns (CS, slots): CS[j] = capacity of class j (>=256);
    slots[core][j] = (expert, lo, hi) token range of that chunk."""
    E = len(counts)
    n_slots = N_CORES * SLOTS_PER_CORE
    k = [1] * E
    while sum(k) < n_slots:
        e = max(range(E), key=lambda i: -(-counts[i] // k[i]))
        k[e] += 1
    chunks = []
    for e in range(E):
        n, parts = counts[e], k[e]
        base, rem = divmod(n, parts)
        lo = 0
        for i in range(parts):
            sz = base + (1 if i < rem else 0)
            chunks.append((sz, e, lo, lo + sz))
            lo += sz
    chunks.sort(key=lambda c: -c[0])
    CS = []
    slots = [[None] * SLOTS_PER_CORE for _ in range(N_CORES)]
    for j in range(SLOTS_PER_CORE):
        cls = chunks[j * N_CORES:(j + 1) * N_CORES]
        sz = cls[0][0]
        CS.append(max(256, sz + (sz & 1)))
        order = sorted(range(N_CORES),
                       key=lambda c: sum(s[2] - s[1] for s in slots[c] if s))
        for i, (sz, e, lo, hi) in enumerate(sorted(cls, key=lambda c: -c[0])):
            slots[order[i]][j] = (e, lo, hi)
    return CS, slots


def _token_chunks(C):
    """Split C into matmul moving-dim chunks, each <=512 (PSUM bank) and as
    equal as possible (>=256 keeps float32r at full rate)."""
    assert C % 2 == 0
    n = -(-C // 512)
    h = C // 2
    base, rem = divmod(h, n)
    sizes = [2 * (base + (1 if i < rem else 0)) for i in range(n)]
    assert sum(sizes) == C and all(s <= 512 and s % 2 == 0 for s in sizes)
    return sizes


# ---------------------------------------------------------------------------
def _build_program(CS, legalize=True):
    """Emit the SPMD Bass program; CS = per-slot-class capacities."""
    S = len(CS)
    nc = bass.Bass()
    xt_d = [nc.dram_tensor(f"XT{s}", [H, CS[s]], F32R, kind="ExternalInput") for s in range(S)]
    w1_d = [nc.dram_tensor(f"W1{s}", [KM, 128, H], F32R, kind="ExternalInput") for s in range(S)]
    w2_d = [nc.dram_tensor(f"W2{s}", [KM, 128, M], F32R, kind="ExternalInput") for s in range(S)]
    w3_d = [nc.dram_tensor(f"W3{s}", [M, H], F32R, kind="ExternalInput") for s in range(S)]
    b1_d = [nc.dram_tensor(f"B1{s}", [M], F32, kind="ExternalInput") for s in range(S)]
    b2_d = [nc.dram_tensor(f"B2{s}", [M], F32, kind="ExternalInput") for s in range(S)]
    cw_d = [nc.dram_tensor(f"CW{s}", [CS[s]], F32, kind="ExternalInput") for s in range(S)]
    y_d = [nc.dram_tensor(f"Y{s}", [CS[s], H], F32, kind="ExternalOutput") for s in range(S)]

    HB = 512 if len(CS) >= 3 else 256

    with tile.TileContext(nc) as tc:
        with (
            tc.tile_pool(name="xt", bufs=1) as p_xt,
            tc.tile_pool(name="w1", bufs=3) as p_w1,
            tc.tile_pool(name="w2", bufs=3) as p_w2,
            tc.tile_pool(name="w3", bufs=8) as p_w3,
            tc.tile_pool(name="h1", bufs=1) as p_h1,
            tc.tile_pool(name="h2", bufs=1) as p_h2,
            tc.tile_pool(name="bias", bufs=1) as p_b,
            tc.tile_pool(name="y", bufs=4) as p_y,
            tc.tile_pool(name="ps", bufs=8, space="PSUM") as p_ps,
        ):
            st = [dict() for _ in range(S)]   # per-slot tiles/geometry

            def emit_loads(s):
                C = CS[s]
                v = st[s]
                v["tcs"] = _token_chunks(C)
                v["tco"] = np.cumsum([0] + v["tcs"]).tolist()
                v["tts"] = [(i * 128, min(128, C - i * 128)) for i in range(-(-C // 128))]
                v["xt"] = [p_xt.tile([128, C], F32R, tag=f"xt{k}", name=f"xt_{s}_{k}") for k in range(KH)]
                for k in range(KH):
                    nc.sync.dma_start(out=v["xt"][k][:], in_=xt_d[s][k * 128:(k + 1) * 128, :])
                v["b1t"] = [p_b.tile([128, 1], F32, tag=f"b1_{s}_{m}", name=f"b1t_{s}_{m}") for m in range(KM)]
                v["b2t"] = [p_b.tile([128, 1], F32, tag=f"b2_{s}_{m}", name=f"b2t_{s}_{m}") for m in range(KM)]
                for m in range(KM):
                    nc.scalar.dma_start(out=v["b1t"][m][:], in_=b1_d[s][m * 128:(m + 1) * 128].rearrange("(p one) -> p one", one=1))
                    nc.scalar.dma_start(out=v["b2t"][m][:], in_=b2_d[s][m * 128:(m + 1) * 128].rearrange("(p one) -> p one", one=1))
                v["cwt"] = []
                for t, (t0, tn) in enumerate(v["tts"]):
                    v["cwt"].append(p_b.tile([tn, 1], F32, tag=f"cw_{s}_{t}", name=f"cwt_{s}_{t}"))
                    nc.scalar.dma_start(out=v["cwt"][t][:], in_=cw_d[s][t0:t0 + tn].rearrange("(p one) -> p one", one=1))
                v["h1"] = [p_h1.tile([128, C], F32R, tag=f"h1_{m}", name=f"h1_{s}_{m}") for m in range(KM)]
                v["h2"] = [p_h2.tile([128, C], F32R, tag=f"h2_{m}", name=f"h2_{s}_{m}") for m in range(KM)]

            def st1_group(s, m):
                v = st[s]
                w1t = p_w1.tile([128, H], F32R, tag="w1", name=f"w1t_{s}_{m}")
                nc.gpsimd.dma_start(out=w1t[:], in_=w1_d[s][m])
                for ci, tcz in enumerate(v["tcs"]):
                    ps = p_ps.tile([128, tcz], F32, tag="ps", name=f"ps1_{s}_{m}_{ci}")
                    for k in range(KH):
                        nc.tensor.matmul(
                            ps[:], w1t[:, k * 128:(k + 1) * 128],
                            v["xt"][k][:, v["tco"][ci]:v["tco"][ci + 1]],
                            start=(k == 0), stop=(k == KH - 1))
                    nc.scalar.activation(
                        v["h1"][m][:, v["tco"][ci]:v["tco"][ci + 1]], ps[:],
                        mybir.ActivationFunctionType.Relu, bias=v["b1t"][m][:, 0:1])

            def st2_group(s, m):
                v = st[s]
                w2t = p_w2.tile([128, M], F32R, tag="w2", name=f"w2t_{s}_{m}")
                nc.gpsimd.dma_start(out=w2t[:], in_=w2_d[s][m])
                for ci, tcz in enumerate(v["tcs"]):
                    ps = p_ps.tile([128, tcz], F32, tag="ps", name=f"ps2_{s}_{m}_{ci}")
                    for k in range(KM):
                        nc.tensor.matmul(
                            ps[:], w2t[:, k * 128:(k + 1) * 128],
                            v["h1"][k][:, v["tco"][ci]:v["tco"][ci + 1]],
                            start=(k == 0), stop=(k == KM - 1))
                    nc.scalar.activation(
                        v["h2"][m][:, v["tco"][ci]:v["tco"][ci + 1]], ps[:],
                        mybir.ActivationFunctionType.Relu, bias=v["b2t"][m][:, 0:1])

            def emit_w3(s, hb):
                w3t = [p_w3.tile([128, HB], F32R, tag=f"w3_{k % 4}", name=f"w3t_{s}_{hb}_{k}") for k in range(KM)]
                for k in range(KM):
                    nc.gpsimd.dma_start(
                        out=w3t[k][:],
                        in_=w3_d[s][k * 128:(k + 1) * 128, hb * HB:(hb + 1) * HB])
                st[s][f"w3_{hb}"] = w3t

            def st3_group(s, hb, t):
                v = st[s]
                t0, tn = v["tts"][t]
                w3t = v[f"w3_{hb}"]
                ps = p_ps.tile([tn, HB], F32, tag="ps", name=f"ps3_{s}_{hb}_{t}")
                for k in range(KM):
                    nc.tensor.matmul(
                        ps[:], v["h2"][k][:, t0:t0 + tn], w3t[k][:],
                        start=(k == 0), stop=(k == KM - 1))
                yt = p_y.tile([tn, HB], F32, tag="y", name=f"yt_{s}_{hb}_{t}")
                nc.scalar.activation(
                    yt[:], ps[:], mybir.ActivationFunctionType.Copy,
                    scale=v["cwt"][t][:, 0:1])
                nc.scalar.dma_start(
                    out=y_d[s][t0:t0 + tn, hb * HB:(hb + 1) * HB],
                    in_=yt[:])

            def st3_emitters(s):
                ems = []
                for hb in range(H // HB):
                    if hb > 0:
                        ems.append(lambda s=s, hb=hb: emit_w3(s, hb))
                    for t in range(len(st[s]["tts"])):
                        ems.append(lambda s=s, hb=hb, t=t: st3_group(s, hb, t))
                return ems

            def interleave(a_ems, b_ems):
                """Emit a and b emitter lists merged evenly (b spread among a)."""
                na, nb = len(a_ems), len(b_ems)
                bi = 0
                for i, a in enumerate(a_ems):
                    while bi < nb and bi * na <= i * nb:
                        b_ems[bi]()
                        bi += 1
                    a()
                while bi < nb:
                    b_ems[bi]()
                    bi += 1

            # ---- emission schedule: st3(s-1) interleaves with st1(s) ----
            emit_loads(0)
            prev_st3 = []
            for s in range(S):
                if s > 0:
                    emit_loads(s)
                interleave([lambda s=s, m=m: st1_group(s, m) for m in range(KM)],
                           prev_st3)
                emit_w3(s, 0)        # prefetch stage-3 hb=0 weights early
                for m in range(KM):
                    st2_group(s, m)
                prev_st3 = st3_emitters(s)
            for em in prev_st3:
                em()

    if legalize:
        _legalize_waits(nc)
    return nc


# ---------------------------------------------------------------------------
def kernel(x, genre_embed, rms_w, wg_W, wg_b, gg_W, gg_b, W1, b1, W2, b2, W3, b3):
    x = np.asarray(x, np.float32)
    B, S_, _ = x.shape
    T = B * S_
    x2d = np.ascontiguousarray(x.reshape(T, H))
    W1 = np.asarray(W1, np.float32)
    W2 = np.asarray(W2, np.float32)
    W3 = np.asarray(W3, np.float32)

    cw = _route(x2d, np.asarray(genre_embed, np.float32), np.asarray(rms_w, np.float32),
                np.asarray(wg_W, np.float32), np.asarray(wg_b, np.float32),
                np.asarray(gg_W, np.float32), np.asarray(gg_b, np.float32), B, S_)
    E = cw.shape[1]
    tok_by_e = [np.nonzero(cw[:, e])[0] for e in range(E)]
    counts = [len(t) for t in tok_by_e]
    CS, slots = _partition(counts)

    # pre-tile weights once per expert (shared across cores)
    used = set(e for core in slots for (e, _, _) in core)
    w1_tiled, w2_tiled = {}, {}
    for e in used:
        w1_tiled[e] = np.ascontiguousarray(
            W1[e].reshape(KH, 128, KM, 128).transpose(2, 1, 0, 3).reshape(KM, 128, H))
        w2_tiled[e] = np.ascontiguousarray(
            W2[e].reshape(KM, 128, KM, 128).transpose(2, 1, 0, 3).reshape(KM, 128, M))

    in_maps = []
    meta = []
    for core in range(N_CORES):
        im = {}
        cmeta = []
        for si, (e, lo, hi) in enumerate(slots[core]):
            C = CS[si]
            idx = tok_by_e[e][lo:hi]
            n = len(idx)
            xt = np.zeros((H, C), np.float32)
            xt[:, :n] = x2d[idx].T
            cwc = np.zeros((C,), np.float32)
            cwc[:n] = cw[idx, e]
            im[f"XT{si}"] = xt
            im[f"W1{si}"] = w1_tiled[e]
            im[f"W2{si}"] = w2_tiled[e]
            im[f"W3{si}"] = W3[e]
            im[f"B1{si}"] = np.asarray(b1[e], np.float32)
            im[f"B2{si}"] = np.asarray(b2[e], np.float32)
            im[f"CW{si}"] = cwc
            cmeta.append(idx)
        in_maps.append(im)
        meta.append(cmeta)

    nc = _build_program(CS)
    res = run_bass_kernel_spmd(nc, in_maps, list(range(N_CORES)))

    out2d = cw @ np.asarray(b3, np.float32)      # bias-3 combine term [T, H]
    for core in range(N_CORES):
        for si, idx in enumerate(meta[core]):
            y = res.results[core][f"Y{si}"]
            out2d[idx] += y[:len(idx)]
    return out2d.reshape(B, S_, H).astype(np.float32)
